# revision 35
# baseline (speedup 1.0000x reference)
"""GQA attention (B=2, T=2048, DIM=2048, NH=32, NKV=8, HD=64) with RoPE, causal,
on 8 TRN2 NeuronCores.

Sharding: data-parallel over B (2) x tensor-parallel over kv-head groups (4).
Core i handles batch i//4 and kv heads {2g, 2g+1} (g = i%4), i.e. q heads
8g..8g+8. wq/wk/wv column-parallel, wo row-parallel; host sums the 4 partial
outputs per batch.

Everything on-device is feature-major ("transposed"): x^T, Q^T, K^T are
[feature, t] so no on-device transposes are needed anywhere:
  QT[d,t] = wq^T x^T;  scoresT[s,q] = (KT slice)^T @ QT;  OT[d,q] = V^T @ PT;
  outT[o,t] = wo^T @ OT.  Host transposes the final [o,t] back to [t,o].

v2 structure (vs baseline):
- Attention inner unit is one 128-row s-block covering BOTH kv halves: the two
  score matmuls (K=64) write cols 0:512 / 512:1024 of one [128,1024] PSUM pair
  and auto-derive PE row-tiles (0,0)/(64,0), so being emission-adjacent they
  run CONCURRENTLY (2x score throughput). One 1024-col exp covers both halves.
- Projection / wo matmuls are interleaved as "filler" into the attention
  emission stream in program order, so the PE works on dense GEMMs while ACT
  streams the exps instead of stalling on the score->exp->PV chain.
- Denominator: ones-column on V gives row 64 of each PV accumulator; the two
  denom rows are DMA-gathered to one [2,512] tile, one reciprocal, gpsimd
  partition-broadcasts, and one multiply per half produce the normalized OTNr.
- ACT exp table preloaded at kernel start (off critical path).

Softmax: no max subtraction (|score| <~ 6 after the 1/8 scale folded into wq).
Causal mask: DVE multiply by a host-precomputed 0/1 tile on diagonal s-blocks
only (mask layout duplicated per half); fully-masked regions are trimmed from
the matmuls.

All matmul operands are fp16 (KERNEL_MM_DTYPE also allows bf16/f32r); fp32
PSUM accumulate.
"""

import numpy as np

B, T, DIM = 2, 2048, 2048
NH, NKV, HD = 32, 8, 64
G = 4            # tensor-parallel groups
QH = NH // G     # 8 local q heads
SLABS = 4
KTILES = DIM // 128
TBLK = T // 512

_CACHE = {}
import os as _os
_MM_DTYPE = _os.environ.get("KERNEL_MM_DTYPE", "fp16")


def _to_mm_dtype(x: np.ndarray) -> np.ndarray:
    if _MM_DTYPE == "bf16":
        import ml_dtypes
        return np.ascontiguousarray(x, dtype=np.float32).astype(ml_dtypes.bfloat16)
    if _MM_DTYPE == "fp16":
        return np.ascontiguousarray(x, dtype=np.float32).astype(np.float16)
    return _round_f32r(x)


def _round_f32r(x: np.ndarray) -> np.ndarray:
    """Round f32 to the float32r grid (11 mantissa bits, round-to-nearest-even)."""
    x = np.ascontiguousarray(x, dtype=np.float32)
    xi = x.view(np.uint32).copy()
    shift = 12  # keep 11 mantissa bits
    lsb = (xi >> shift) & 1
    xi = (xi + ((1 << (shift - 1)) - 1) + lsb) & np.uint32(~((1 << shift) - 1) & 0xFFFFFFFF)
    return xi.view(np.float32)


def _build():
    import concourse.bass as bass
    import concourse.mybir as mybir
    import concourse.tile as tile
    from concourse import bacc

    F32 = mybir.dt.float32
    F32R = {"bf16": mybir.dt.bfloat16, "fp16": mybir.dt.float16,
            "f32r": mybir.dt.float32r}[_MM_DTYPE]
    EXP = mybir.ActivationFunctionType.Exp

    nc = bacc.Bacc("TRN2", target_bir_lowering=False, debug=False, num_devices=8)

    xT = nc.dram_tensor("xT", [DIM, T], F32R, kind="ExternalInput").ap()
    # weights arrive host-pre-arranged to the SBUF layout [p, k*C + c] so the
    # loads are contiguous 128-row DMAs instead of thousands of tiny
    # rearrange descriptors (the old WK/WV triggers alone cost ~1.4us each
    # and delayed the first matmul to ~17us)
    wq = nc.dram_tensor("wq", [128, KTILES * QH * HD], F32R, kind="ExternalInput").ap()
    wk = nc.dram_tensor("wk", [128, KTILES * 2 * HD], F32R, kind="ExternalInput").ap()
    wv = nc.dram_tensor("wv", [128, KTILES * 2 * HD], F32R, kind="ExternalInput").ap()
    wo = nc.dram_tensor("wo", [QH * HD, DIM], F32R, kind="ExternalInput").ap()
    c4 = nc.dram_tensor("c4", [128, T], F32R, kind="ExternalInput").ap()
    s4 = nc.dram_tensor("s4", [128, T], F32R, kind="ExternalInput").ap()
    vones = nc.dram_tensor("vones", [128, 16 * 65], F32R, kind="ExternalInput").ap()
    msk = nc.dram_tensor("msk", [128, 4096], F32R, kind="ExternalInput").ap()
    outT = nc.dram_tensor("outT", [DIM, T], F32R, kind="ExternalOutput").ap()

    from contextlib import ExitStack

    with tile.TileContext(nc) as tc, ExitStack() as ctx:
        # ---------- persistent tiles ----------
        pers = ctx.enter_context(tc.tile_pool(name="pers", bufs=1))
        KT = pers.tile([128, T], F32R, tag="kt", name="kt")
        V0 = pers.tile([128, 16 * 65], F32R, tag="v0", name="v0")
        V1 = pers.tile([128, 16 * 65], F32R, tag="v1", name="v1")
        MSK = pers.tile([128, 4096], F32R, tag="msk", name="msk_sb")
        C4 = pers.tile([128, T], F32R, tag="c4", name="c4_sb")
        S4 = pers.tile([128, T], F32R, tag="s4", name="s4_sb")
        WQ = pers.tile([128, KTILES * 512], F32R, tag="wq", name="wq_sb")
        WK = pers.tile([128, KTILES * 128], F32R, tag="wk", name="wk_sb")
        WV = pers.tile([128, KTILES * 128], F32R, tag="wv", name="wv_sb")
        WO = [pers.tile([128, T], F32R, tag=f"wo{s}", name=f"wo{s}") for s in range(SLABS)]

        rot = ctx.enter_context(tc.tile_pool(name="rot", bufs=3))
        work = ctx.enter_context(tc.tile_pool(name="work", bufs=3))
        ptp = ctx.enter_context(tc.tile_pool(name="ptp", bufs=6))
        misc = ctx.enter_context(tc.tile_pool(name="misc", bufs=2))
        osbp = ctx.enter_context(tc.tile_pool(name="osbp", bufs=6))
        xtp = ctx.enter_context(tc.tile_pool(name="xt", bufs=2))
        ps_acc = ctx.enter_context(tc.tile_pool(name="ps_acc", bufs=2, space="PSUM"))
        ps_sc = ctx.enter_context(tc.tile_pool(name="ps_sc", bufs=2, space="PSUM"))
        ps_ot = ctx.enter_context(tc.tile_pool(name="ps_ot", bufs=1, space="PSUM"))

        # ---------- initial DMAs ----------
        def load_xts(tb, names):
            t_sl = slice(tb * 512, (tb + 1) * 512)
            xts = []
            src = xT.rearrange("(k p) t -> p k t", p=128)[:, :, t_sl]
            for k in range(KTILES):
                xts.append(xtp.tile([128, 512], F32R, tag=f"x{k}",
                                    name=f"{names}_{k}"))
            for k in range(KTILES):
                # alternate trigger queues: 16 descriptors-gen at ~0.6us each
                # would serialize ~10us on one queue
                eng = nc.sync if k % 2 == 0 else nc.gpsimd
                eng.dma_start(xts[k][:], src[:, k])
            return xts

        xts_by_tb = {0: load_xts(0, "xt0")}
        # small operands needed by tb0's K-proj / attention go before the 2MB WQ
        nc.gpsimd.dma_start(WK[:], wk)
        nc.gpsimd.dma_start(WV[:], wv)
        nc.gpsimd.dma_start(C4[:, 0:512], c4[:, 0:512])
        nc.gpsimd.dma_start(S4[:, 0:512], s4[:, 0:512])
        nc.gpsimd.dma_start(V0[:], vones[:])
        nc.gpsimd.dma_start(V1[:], vones[:])
        nc.gpsimd.dma_start(MSK[:], msk[:])
        for c in range(4):
            nc.scalar.dma_start(WQ[:, c * 2048:(c + 1) * 2048],
                                wq[:, c * 2048:(c + 1) * 2048])
        nc.gpsimd.dma_start(C4[:, 512:2048], c4[:, 512:2048])
        nc.gpsimd.dma_start(S4[:, 512:2048], s4[:, 512:2048])
        for s in range(SLABS):
            nc.scalar.dma_start(WO[s][:], wo[s * 128:(s + 1) * 128, :])

        # preload the exp table set while DMAs stream (first real exp would
        # otherwise pay the ~2.7us ACT_TABLE_LOAD on the critical path)
        warm = misc.tile([1, 8], F32, tag="warm", name="warm")
        nc.scalar.activation(warm[:], C4[0:1, 0:8], EXP)

        # ---------- emission machinery: labeled filler queue ----------
        QTr_by = {}    # (tb, s) -> QTr tile, written when the Q-proj item pops
        OTNr_by = {}   # (tb, s) -> OTNr tile, written by emit_att_slab

        filler_q = []            # list of (label, fn)
        label_left = {}          # label -> count not yet emitted

        def fill_append(label, fn):
            filler_q.append((label, fn))
            label_left[label] = label_left.get(label, 0) + 1

        def _pop_one():
            label, fn = filler_q.pop(0)
            label_left[label] -= 1
            fn()

        def flush_label(label):
            while label_left.get(label, 0) > 0:
                _pop_one()

        def take_fill(units_left):
            k = (len(filler_q) + units_left - 1) // units_left if units_left > 0 else len(filler_q)
            k = min(k, 4, len(filler_q))
            for _ in range(k):
                _pop_one()

        def flush_all():
            while filler_q:
                _pop_one()

        # ---------- projection emission ----------
        def rope_tail(ps, dst, t_sl):
            """dst = ps*C4 + swap(ps)*S4 (swap via gpsimd DMA). ps is a psum tile."""
            q_sb = work.tile([128, 512], F32, tag="qsb", name="qsb")
            nc.vector.tensor_copy(q_sb[:], ps[:])
            q_sw = work.tile([128, 512], F32, tag="qsw", name="qsw")
            for o in (0, 64):
                nc.gpsimd.dma_start(q_sw[o:o + 32, :], q_sb[o + 32:o + 64, :])
                nc.gpsimd.dma_start(q_sw[o + 32:o + 64, :], q_sb[o:o + 32, :])
            m1 = work.tile([128, 512], F32, tag="m1", name="m1")
            nc.vector.tensor_mul(m1[:], ps[:], C4[:, t_sl])
            m2 = work.tile([128, 512], F32, tag="m2", name="m2")
            nc.vector.tensor_mul(m2[:], q_sw[:], S4[:, t_sl])
            nc.vector.tensor_add(dst, m1[:], m2[:])

        def append_proj_items(tb):
            """Queue tb's x loads + K/V/Q projection groups as filler items."""
            t_sl = slice(tb * 512, (tb + 1) * 512)

            def do_load_x():
                xts_by_tb[tb] = load_xts(tb, f"xt{tb}")
            if tb not in xts_by_tb:
                fill_append(("X", tb), do_load_x)

            # K projection: 16 accumulating MMs in 4 chunks + rope into KT
            kps = {}
            def k_chunk(c):
                def fn():
                    if c == 0:
                        kps["ps"] = ps_acc.tile([128, 512], mybir.dt.float32,
                                                tag="acc", name="pk")
                    ps = kps["ps"]
                    xts = xts_by_tb[tb]
                    for k in range(c * 4, c * 4 + 4):
                        nc.tensor.matmul(ps[:], WK[:, k * 128:(k + 1) * 128],
                                         xts[k][:], start=(k == 0),
                                         stop=(k == KTILES - 1))
                    if c == 3:
                        rope_tail(ps, KT[:, t_sl], t_sl)
                return fn

            # Q projections: per slab 16 MMs in 4 chunks + rope into QTr
            def make_q_chunk(s):
                qps = {}
                def q_chunk(c):
                    def fn():
                        if c == 0:
                            qps["ps"] = ps_acc.tile([128, 512], mybir.dt.float32,
                                                    tag="acc", name="pq")
                        ps = qps["ps"]
                        xts = xts_by_tb[tb]
                        for k in range(c * 4, c * 4 + 4):
                            nc.tensor.matmul(
                                ps[:], WQ[:, k * 512 + s * 128: k * 512 + (s + 1) * 128],
                                xts[k][:], start=(k == 0), stop=(k == KTILES - 1))
                        if c == 3:
                            dst_t = rot.tile([128, 512], F32R, tag=f"qtr{s}",
                                             name=f"qtr{s}")
                            QTr_by[(tb, s)] = dst_t
                            rope_tail(ps, dst_t[:], t_sl)
                    return fn
                return q_chunk

            if tb == 0:
                # head: interleave K and Q0 chunks so PE can start as soon as
                # the first x k-tiles land instead of waiting for all 16
                q0 = make_q_chunk(0)
                for c in range(4):
                    fill_append(("K", tb), k_chunk(c))
                    fill_append(("Q", tb, 0), q0(c))
            else:
                for c in range(4):
                    fill_append(("K", tb), k_chunk(c))

            # V projection: 4 items (one per 128-t block)
            def v_item(i):
                def fn():
                    sbi = tb * 4 + i
                    xts = xts_by_tb[tb]
                    pv = ps_acc.tile([128, 128], mybir.dt.float32, tag="acc",
                                     name="pv", padded_shape=[128, 512])
                    for k in range(KTILES):
                        nc.tensor.matmul(pv[:], xts[k][:, i * 128:(i + 1) * 128],
                                         WV[:, k * 128:(k + 1) * 128],
                                         start=(k == 0), stop=(k == KTILES - 1))
                    nc.vector.tensor_copy(V0[:, sbi * 65: sbi * 65 + 64], pv[:, 0:64])
                    nc.vector.tensor_copy(V1[:, sbi * 65: sbi * 65 + 64], pv[:, 64:128])
                return fn
            for i in range(4):
                fill_append(("V", tb), v_item(i))

            for s in range(0 if tb != 0 else 1, SLABS):
                qc_ = make_q_chunk(s)
                for c in range(4):
                    fill_append(("Q", tb, s), qc_(c))

        def append_wo_items(tb, obs, cast_on_act=False):
            """Queue some of tb's output projection ob-groups as filler items.

            cast_on_act routes the PSUM->SBUF cast to the Scalar engine for
            windows where ACT has slack (early tbs, tail) and DVE does not.
            """
            t_sl = slice(tb * 512, (tb + 1) * 512)
            def wo_item(ob):
                def fn():
                    po = ps_acc.tile([128, 512], mybir.dt.float32, tag="acc",
                                     name="po")
                    for s in range(SLABS):
                        nc.tensor.matmul(po[:], WO[s][:, ob * 128:(ob + 1) * 128],
                                         OTNr_by[(tb, s)][:],
                                         start=(s == 0), stop=(s == SLABS - 1))
                    osb = osbp.tile([128, 512], F32R, tag="osb", name="osb")
                    if cast_on_act:
                        nc.scalar.copy(osb[:], po[:])
                    else:
                        nc.vector.tensor_copy(osb[:], po[:])
                    # keep the sync queue free for x loads
                    nc.gpsimd.dma_start(outT[ob * 128:(ob + 1) * 128, t_sl], osb[:])
                return fn
            for ob in obs:
                fill_append(("wo", tb), wo_item(ob))

        # ---------- attention ----------
        def emit_att_slab(tb, s, units_left_ref):
            qc = tb
            nblk = 4 * qc + 4
            QTr_s = QTr_by[(tb, s)]
            ot0 = ps_ot.tile([65, 512], mybir.dt.float32, tag="ot0", name="ot0")
            ot1 = ps_ot.tile([65, 512], mybir.dt.float32, tag="ot1", name="ot1")
            pend = None   # (pt, o, b) awaiting PV emission

            def emit_pv(pt, o, b):
                nc.tensor.matmul(ot0[:, o:512], V0[:, b * 65: b * 65 + 65],
                                 pt[:, o:512],
                                 start=(b == 0), stop=(b == nblk - 1))
                nc.tensor.matmul(ot1[:, o:512], V1[:, b * 65: b * 65 + 65],
                                 pt[:, 512 + o:1024],
                                 start=(b == 0), stop=(b == nblk - 1))

            pend = []  # PV emission lags 2 units behind the score/exp stream
            for b in range(nblk):
                o = max(0, (b - 4 * qc) * 128)
                sc = ps_sc.tile([128, 1024], mybir.dt.float32, tag="sc", name="sc")
                # both halves of this s-block: row-tiles (0,0)/(64,0), adjacent
                # in PE order -> concurrent
                nc.tensor.matmul(sc[:, o:512], KT[0:64, b * 128:(b + 1) * 128],
                                 QTr_s[0:64, o:512], start=True, stop=True)
                nc.tensor.matmul(sc[:, 512 + o:1024], KT[64:128, b * 128:(b + 1) * 128],
                                 QTr_s[64:128, o:512], start=True, stop=True)
                pt = ptp.tile([128, 1024], F32R, tag="pt", name="pt")
                nc.scalar.activation(pt[:, o:1024], sc[:, o:1024], EXP)
                if b >= 4 * qc:  # diagonal block: zero the intra-block triangle
                    v = b - 4 * qc
                    nc.vector.tensor_mul(pt[:, o:1024], pt[:, o:1024],
                                         MSK[:, v * 1024 + o:(v + 1) * 1024])
                units_left_ref[0] -= 1
                pend.append((pt, o, b))
                # batch PVs in pairs so the PE sees 4-MM runs of each tiling
                # mode (scores 64-row, PV 128-row) -> half the mode switches
                if b % 2 == 1:
                    take_fill(max(1, units_left_ref[0] // 2))
                    if len(pend) >= 4:
                        for item in (pend.pop(0), pend.pop(0)):
                            if item[2] == 0:
                                flush_label(("V", tb))
                            emit_pv(*item)
            for item in pend:
                if item[2] == 0:
                    flush_label(("V", tb))
                emit_pv(*item)

            # ---- denominators + normalize: evacuate each PSUM accumulator to
            # SBUF with one copy (frees the ot slot for the next slab right
            # away), then run the recip/broadcast/normalize off the copy ----
            OTNr_s = rot.tile([128, 512], F32R, tag=f"otnr{s}", name=f"otnr{s}")
            OTNr_by[(tb, s)] = OTNr_s
            for h, oth in ((0, ot0), (1, ot1)):
                d = misc.tile([1, 512], F32, tag=f"d{h}", name=f"d{h}")
                nc.vector.tensor_copy(d[:], oth[64:65, :])
                osb_h = misc.tile([64, 512], F32, tag=f"osb{h}", name=f"osb{h}")
                nc.vector.tensor_copy(osb_h[:], oth[0:64, :])
                r = misc.tile([1, 512], F32, tag=f"r{h}", name=f"r{h}")
                nc.vector.reciprocal_approx_fast(r[:], d[:])
                bch = misc.tile([64, 512], F32, tag=f"bc{h}", name=f"bc{h}")
                nc.gpsimd.partition_broadcast(bch[:], r[:])
                nc.vector.tensor_mul(OTNr_s[h * 64:(h + 1) * 64, :],
                                     osb_h[:], bch[:])

        # ---------- main loop ----------
        # wo(tb) is split 8/8 across the next two tb windows so the exp-heavy
        # late chunks keep enough PE filler
        append_proj_items(0)
        for tb in range(TBLK):
            if tb >= 2:
                # pops first in this tb, freeing OTNr(tb-2) slots early
                append_wo_items(tb - 2, range(8, 16), cast_on_act=(tb <= 2))
            if tb == TBLK - 1:
                append_wo_items(tb - 1, range(8, 16))
            if tb + 1 < TBLK:
                append_proj_items(tb + 1)
            units_left = [4 * (4 * tb + 4)]
            for s in range(SLABS):
                flush_label(("X", tb))
                flush_label(("K", tb))
                flush_label(("V", tb))
                flush_label(("Q", tb, s))
                emit_att_slab(tb, s, units_left)
            # popped during tb+1 (or the tail for the last tb): ACT has slack
            # there except during tb3's exp-heavy attention
            append_wo_items(tb, range(0, 8),
                            cast_on_act=(tb <= 1 or tb == TBLK - 1))
        append_wo_items(TBLK - 1, range(8, 16), cast_on_act=True)
        flush_all()

    nc.compile()
    return nc


def _prep_inputs(x, freqs_cos, freqs_sin, wq, wk, wv, wo):
    """Build the 8 per-core input maps (host-side sharding + layout prep)."""
    x = np.asarray(x, dtype=np.float32)
    freqs_cos = np.asarray(freqs_cos, dtype=np.float32)
    freqs_sin = np.asarray(freqs_sin, dtype=np.float32)
    wq = np.asarray(wq, dtype=np.float32)
    wk = np.asarray(wk, dtype=np.float32)
    wv = np.asarray(wv, dtype=np.float32)
    wo = np.asarray(wo, dtype=np.float32)

    # de-interleave permutation within a head: [2j] then [2j+1]
    deint = np.concatenate([np.arange(0, HD, 2), np.arange(1, HD, 2)])

    # rope tables [128, T]: row r uses freq index r % 32; sign of sin flips
    # per 32-block (real-out blocks get -sin)
    cosT = freqs_cos.T  # [32, T]
    sinT = freqs_sin.T
    c4 = np.tile(cosT, (4, 1)).astype(np.float32)
    s4 = np.concatenate([-sinT, sinT, -sinT, sinT], axis=0).astype(np.float32)

    vones = np.zeros((128, 16 * 65), dtype=np.float32)
    vones[:, 64::65] = 1.0
    # block masks: variant v covers diagonal s-block at offset 128v vs q in
    # [0,512); duplicated for the two halves (cols 0:512 and 512:1024):
    # msk[p, v*1024 + h*512 + q] = 1 if (128v + p) <= q else 0
    msk = np.zeros((128, 4096), dtype=np.float32)
    p_ = np.arange(128)[:, None]
    q_ = np.arange(512)[None, :]
    for v in range(4):
        blk = (128 * v + p_) <= q_
        for h in range(2):
            msk[:, v * 1024 + h * 512:(v * 1024 + (h + 1) * 512)] = blk

    in_maps = []
    for core in range(8):
        b, g = divmod(core, 4)
        # local q head order: slab-major, (s, half) -> global head 8g + s + 4*half
        qheads = [8 * g + s + 4 * h for s in range(SLABS) for h in range(2)]
        kvheads = [2 * g, 2 * g + 1]

        wq_cols = np.concatenate([qh * HD + deint for qh in qheads])
        wk_cols = np.concatenate([kh * HD + deint for kh in kvheads])
        wv_cols = np.concatenate([np.arange(kh * HD, (kh + 1) * HD) for kh in kvheads])
        wo_rows = np.concatenate([np.arange(qh * HD, (qh + 1) * HD) for qh in qheads])

        def prearr(w):
            # [DIM, C] -> [128, KTILES*C]: element [p, k*C+c] = w[k*128+p, c],
            # matching the SBUF layout so the device DMA is contiguous
            C = w.shape[1]
            return w.reshape(KTILES, 128, C).transpose(1, 0, 2).reshape(128, KTILES * C)

        in_maps.append({
            "xT": _to_mm_dtype(x[b].T),
            "wq": _to_mm_dtype(prearr(wq[:, wq_cols] * (1.0 / np.sqrt(HD)))),
            "wk": _to_mm_dtype(prearr(wk[:, wk_cols])),
            "wv": _to_mm_dtype(prearr(wv[:, wv_cols])),
            "wo": _to_mm_dtype(wo[wo_rows, :]),
            "c4": _to_mm_dtype(c4),
            "s4": _to_mm_dtype(s4),
            "vones": _to_mm_dtype(vones),
            "msk": _to_mm_dtype(msk),
        })
    return in_maps


def kernel(x, freqs_cos, freqs_sin, wq, wk, wv, wo, _trace=False):
    from concourse.bass_utils import run_bass_kernel_spmd

    if "nc" not in _CACHE:
        _CACHE["nc"] = _build()
    nc = _CACHE["nc"]

    in_maps = _prep_inputs(x, freqs_cos, freqs_sin, wq, wk, wv, wo)
    res = run_bass_kernel_spmd(nc, in_maps, core_ids=list(range(8)), trace=_trace)
    _CACHE["last_result"] = res

    out = np.empty((B, T, DIM), dtype=np.float32)
    for b in range(B):
        acc = res.results[4 * b]["outT"].astype(np.float32)
        for g in range(1, 4):
            acc = acc + res.results[4 * b + g]["outT"].astype(np.float32)
        out[b] = acc.T
    return out


# revision 40
# speedup vs baseline: 1.0037x; 1.0037x over previous
"""GQA attention (B=2, T=2048, DIM=2048, NH=32, NKV=8, HD=64) with RoPE, causal,
on 8 TRN2 NeuronCores.

Sharding: data-parallel over B (2) x tensor-parallel over kv-head groups (4).
Core i handles batch i//4 and kv heads {2g, 2g+1} (g = i%4), i.e. q heads
8g..8g+8. wq/wk/wv column-parallel, wo row-parallel; host sums the 4 partial
outputs per batch.

Everything on-device is feature-major ("transposed"): x^T, Q^T, K^T are
[feature, t] so no on-device transposes are needed anywhere:
  QT[d,t] = wq^T x^T;  scoresT[s,q] = (KT slice)^T @ QT;  OT[d,q] = V^T @ PT;
  outT[o,t] = wo^T @ OT.  Host transposes the final [o,t] back to [t,o].

v2 structure (vs baseline):
- Attention inner unit is one 128-row s-block covering BOTH kv halves: the two
  score matmuls (K=64) write cols 0:512 / 512:1024 of one [128,1024] PSUM pair
  and auto-derive PE row-tiles (0,0)/(64,0), so being emission-adjacent they
  run CONCURRENTLY (2x score throughput). One 1024-col exp covers both halves.
- Projection / wo matmuls are interleaved as "filler" into the attention
  emission stream in program order, so the PE works on dense GEMMs while ACT
  streams the exps instead of stalling on the score->exp->PV chain.
- Denominator: ones-column on V gives row 64 of each PV accumulator; the two
  denom rows are DMA-gathered to one [2,512] tile, one reciprocal, gpsimd
  partition-broadcasts, and one multiply per half produce the normalized OTNr.
- ACT exp table preloaded at kernel start (off critical path).

Softmax: no max subtraction (|score| <~ 6 after the 1/8 scale folded into wq).
Causal mask: DVE multiply by a host-precomputed 0/1 tile on diagonal s-blocks
only (mask layout duplicated per half); fully-masked regions are trimmed from
the matmuls.

All matmul operands are fp16 (KERNEL_MM_DTYPE also allows bf16/f32r); fp32
PSUM accumulate.
"""

import numpy as np

B, T, DIM = 2, 2048, 2048
NH, NKV, HD = 32, 8, 64
G = 4            # tensor-parallel groups
QH = NH // G     # 8 local q heads
SLABS = 4
KTILES = DIM // 128
TBLK = T // 512

_CACHE = {}
import os as _os
_MM_DTYPE = _os.environ.get("KERNEL_MM_DTYPE", "fp16")


def _to_mm_dtype(x: np.ndarray) -> np.ndarray:
    if _MM_DTYPE == "bf16":
        import ml_dtypes
        return np.ascontiguousarray(x, dtype=np.float32).astype(ml_dtypes.bfloat16)
    if _MM_DTYPE == "fp16":
        return np.ascontiguousarray(x, dtype=np.float32).astype(np.float16)
    return _round_f32r(x)


def _round_f32r(x: np.ndarray) -> np.ndarray:
    """Round f32 to the float32r grid (11 mantissa bits, round-to-nearest-even)."""
    x = np.ascontiguousarray(x, dtype=np.float32)
    xi = x.view(np.uint32).copy()
    shift = 12  # keep 11 mantissa bits
    lsb = (xi >> shift) & 1
    xi = (xi + ((1 << (shift - 1)) - 1) + lsb) & np.uint32(~((1 << shift) - 1) & 0xFFFFFFFF)
    return xi.view(np.float32)


def _build():
    import concourse.bass as bass
    import concourse.mybir as mybir
    import concourse.tile as tile
    from concourse import bacc

    F32 = mybir.dt.float32
    F32R = {"bf16": mybir.dt.bfloat16, "fp16": mybir.dt.float16,
            "f32r": mybir.dt.float32r}[_MM_DTYPE]
    EXP = mybir.ActivationFunctionType.Exp

    nc = bacc.Bacc("TRN2", target_bir_lowering=False, debug=False, num_devices=8)

    xT = nc.dram_tensor("xT", [DIM, T], F32R, kind="ExternalInput").ap()
    # weights arrive host-pre-arranged to the SBUF layout [p, k*C + c] so the
    # loads are contiguous 128-row DMAs instead of thousands of tiny
    # rearrange descriptors (the old WK/WV triggers alone cost ~1.4us each
    # and delayed the first matmul to ~17us)
    wq = nc.dram_tensor("wq", [128, KTILES * QH * HD], F32R, kind="ExternalInput").ap()
    wk = nc.dram_tensor("wk", [128, KTILES * 2 * HD], F32R, kind="ExternalInput").ap()
    wv = nc.dram_tensor("wv", [128, KTILES * 2 * HD], F32R, kind="ExternalInput").ap()
    wo = nc.dram_tensor("wo", [QH * HD, DIM], F32R, kind="ExternalInput").ap()
    c4 = nc.dram_tensor("c4", [128, T], F32R, kind="ExternalInput").ap()
    s4 = nc.dram_tensor("s4", [128, T], F32R, kind="ExternalInput").ap()
    vones = nc.dram_tensor("vones", [128, 16 * 65], F32R, kind="ExternalInput").ap()
    msk = nc.dram_tensor("msk", [128, 4096], F32R, kind="ExternalInput").ap()
    outT = nc.dram_tensor("outT", [DIM, T], F32R, kind="ExternalOutput").ap()

    from contextlib import ExitStack

    with tile.TileContext(nc) as tc, ExitStack() as ctx:
        # ---------- persistent tiles ----------
        pers = ctx.enter_context(tc.tile_pool(name="pers", bufs=1))
        KT = pers.tile([128, T], F32R, tag="kt", name="kt")
        V0 = pers.tile([128, 16 * 65], F32R, tag="v0", name="v0")
        V1 = pers.tile([128, 16 * 65], F32R, tag="v1", name="v1")
        MSK = pers.tile([128, 4096], F32R, tag="msk", name="msk_sb")
        C4 = pers.tile([128, T], F32R, tag="c4", name="c4_sb")
        S4 = pers.tile([128, T], F32R, tag="s4", name="s4_sb")
        WQ = pers.tile([128, KTILES * 512], F32R, tag="wq", name="wq_sb")
        WK = pers.tile([128, KTILES * 128], F32R, tag="wk", name="wk_sb")
        WV = pers.tile([128, KTILES * 128], F32R, tag="wv", name="wv_sb")
        WO = [pers.tile([128, T], F32R, tag=f"wo{s}", name=f"wo{s}") for s in range(SLABS)]

        rot = ctx.enter_context(tc.tile_pool(name="rot", bufs=3))
        work = ctx.enter_context(tc.tile_pool(name="work", bufs=3))
        ptp = ctx.enter_context(tc.tile_pool(name="ptp", bufs=6))
        misc = ctx.enter_context(tc.tile_pool(name="misc", bufs=2))
        osbp = ctx.enter_context(tc.tile_pool(name="osbp", bufs=6))
        xtp = ctx.enter_context(tc.tile_pool(name="xt", bufs=2))
        ps_acc = ctx.enter_context(tc.tile_pool(name="ps_acc", bufs=2, space="PSUM"))
        ps_sc = ctx.enter_context(tc.tile_pool(name="ps_sc", bufs=2, space="PSUM"))
        ps_ot = ctx.enter_context(tc.tile_pool(name="ps_ot", bufs=1, space="PSUM"))

        # ---------- initial DMAs ----------
        def load_xts(tb, names):
            t_sl = slice(tb * 512, (tb + 1) * 512)
            xts = []
            src = xT.rearrange("(k p) t -> p k t", p=128)[:, :, t_sl]
            for k in range(KTILES):
                xts.append(xtp.tile([128, 512], F32R, tag=f"x{k}",
                                    name=f"{names}_{k}"))
            for k in range(KTILES):
                nc.sync.dma_start(xts[k][:], src[:, k])
            return xts

        xts_by_tb = {0: load_xts(0, "xt0")}
        # small operands needed by tb0's K-proj / attention go before the 2MB WQ
        nc.gpsimd.dma_start(WK[:], wk)
        nc.gpsimd.dma_start(WV[:], wv)
        nc.gpsimd.dma_start(C4[:, 0:512], c4[:, 0:512])
        nc.gpsimd.dma_start(S4[:, 0:512], s4[:, 0:512])
        nc.gpsimd.dma_start(V0[:], vones[:])
        nc.gpsimd.dma_start(V1[:], vones[:])
        nc.gpsimd.dma_start(MSK[:], msk[:])
        for c in range(4):
            nc.scalar.dma_start(WQ[:, c * 2048:(c + 1) * 2048],
                                wq[:, c * 2048:(c + 1) * 2048])
        nc.gpsimd.dma_start(C4[:, 512:2048], c4[:, 512:2048])
        nc.gpsimd.dma_start(S4[:, 512:2048], s4[:, 512:2048])
        for s in range(SLABS):
            nc.scalar.dma_start(WO[s][:], wo[s * 128:(s + 1) * 128, :])

        # preload the exp table set while DMAs stream (first real exp would
        # otherwise pay the ~2.7us ACT_TABLE_LOAD on the critical path)
        warm = misc.tile([1, 8], F32, tag="warm", name="warm")
        nc.scalar.activation(warm[:], C4[0:1, 0:8], EXP)

        # ---------- emission machinery: labeled filler queue ----------
        QTr_by = {}    # (tb, s) -> QTr tile, written when the Q-proj item pops
        OTNr_by = {}   # (tb, s) -> OTNr tile, written by emit_att_slab

        filler_q = []            # list of (label, fn)
        label_left = {}          # label -> count not yet emitted

        def fill_append(label, fn):
            filler_q.append((label, fn))
            label_left[label] = label_left.get(label, 0) + 1

        def _pop_one():
            label, fn = filler_q.pop(0)
            label_left[label] -= 1
            fn()

        def flush_label(label):
            while label_left.get(label, 0) > 0:
                _pop_one()

        def take_fill(units_left):
            k = (len(filler_q) + units_left - 1) // units_left if units_left > 0 else len(filler_q)
            k = min(k, 4, len(filler_q))
            for _ in range(k):
                _pop_one()

        def flush_all():
            while filler_q:
                _pop_one()

        # ---------- projection emission ----------
        def rope_tail(ps, dst, t_sl):
            """dst = ps*C4 + swap(ps)*S4 (swap via gpsimd DMA). ps is a psum tile."""
            q_sb = work.tile([128, 512], F32, tag="qsb", name="qsb")
            nc.vector.tensor_copy(q_sb[:], ps[:])
            q_sw = work.tile([128, 512], F32, tag="qsw", name="qsw")
            for o in (0, 64):
                nc.gpsimd.dma_start(q_sw[o:o + 32, :], q_sb[o + 32:o + 64, :])
                nc.gpsimd.dma_start(q_sw[o + 32:o + 64, :], q_sb[o:o + 32, :])
            m1 = work.tile([128, 512], F32, tag="m1", name="m1")
            nc.vector.tensor_mul(m1[:], ps[:], C4[:, t_sl])
            m2 = work.tile([128, 512], F32, tag="m2", name="m2")
            nc.vector.tensor_mul(m2[:], q_sw[:], S4[:, t_sl])
            nc.vector.tensor_add(dst, m1[:], m2[:])

        def append_proj_items(tb):
            """Queue tb's x loads + K/V/Q projection groups as filler items."""
            t_sl = slice(tb * 512, (tb + 1) * 512)

            def do_load_x():
                xts_by_tb[tb] = load_xts(tb, f"xt{tb}")
            if tb not in xts_by_tb:
                fill_append(("X", tb), do_load_x)

            # K projection: 16 accumulating MMs in chunks + rope into KT.
            # tb0 uses fine 4-MM chunks (head is DMA-feed paced); later tbs
            # use 8-MM chunks so fewer filler boundaries expose LDW/drains.
            csz = 4 if tb == 0 else 8
            nch = KTILES // csz
            kps = {}
            def k_chunk(c):
                def fn():
                    if c == 0:
                        kps["ps"] = ps_acc.tile([128, 512], mybir.dt.float32,
                                                tag="acc", name="pk")
                    ps = kps["ps"]
                    xts = xts_by_tb[tb]
                    for k in range(c * csz, (c + 1) * csz):
                        nc.tensor.matmul(ps[:], WK[:, k * 128:(k + 1) * 128],
                                         xts[k][:], start=(k == 0),
                                         stop=(k == KTILES - 1))
                    if c == nch - 1:
                        rope_tail(ps, KT[:, t_sl], t_sl)
                return fn

            # Q projections: per slab 16 MMs in chunks + rope into QTr
            def make_q_chunk(s):
                qps = {}
                def q_chunk(c):
                    def fn():
                        if c == 0:
                            qps["ps"] = ps_acc.tile([128, 512], mybir.dt.float32,
                                                    tag="acc", name="pq")
                        ps = qps["ps"]
                        xts = xts_by_tb[tb]
                        for k in range(c * csz, (c + 1) * csz):
                            nc.tensor.matmul(
                                ps[:], WQ[:, k * 512 + s * 128: k * 512 + (s + 1) * 128],
                                xts[k][:], start=(k == 0), stop=(k == KTILES - 1))
                        if c == nch - 1:
                            dst_t = rot.tile([128, 512], F32R, tag=f"qtr{s}",
                                             name=f"qtr{s}")
                            QTr_by[(tb, s)] = dst_t
                            rope_tail(ps, dst_t[:], t_sl)
                    return fn
                return q_chunk

            if tb == 0:
                # head: interleave K and Q0 chunks so PE can start as soon as
                # the first x k-tiles land instead of waiting for all 16
                q0 = make_q_chunk(0)
                for c in range(nch):
                    fill_append(("K", tb), k_chunk(c))
                    fill_append(("Q", tb, 0), q0(c))
            else:
                for c in range(nch):
                    fill_append(("K", tb), k_chunk(c))

            # V projection: 4 items (one per 128-t block)
            def v_item(i):
                def fn():
                    sbi = tb * 4 + i
                    xts = xts_by_tb[tb]
                    pv = ps_acc.tile([128, 128], mybir.dt.float32, tag="acc",
                                     name="pv", padded_shape=[128, 512])
                    for k in range(KTILES):
                        nc.tensor.matmul(pv[:], xts[k][:, i * 128:(i + 1) * 128],
                                         WV[:, k * 128:(k + 1) * 128],
                                         start=(k == 0), stop=(k == KTILES - 1))
                    nc.vector.tensor_copy(V0[:, sbi * 65: sbi * 65 + 64], pv[:, 0:64])
                    nc.vector.tensor_copy(V1[:, sbi * 65: sbi * 65 + 64], pv[:, 64:128])
                return fn
            for i in range(4):
                fill_append(("V", tb), v_item(i))

            for s in range(0 if tb != 0 else 1, SLABS):
                qc_ = make_q_chunk(s)
                for c in range(nch):
                    fill_append(("Q", tb, s), qc_(c))

        def append_wo_items(tb, obs, cast_on_act=False):
            """Queue some of tb's output projection ob-groups as filler items.

            cast_on_act routes the PSUM->SBUF cast to the Scalar engine for
            windows where ACT has slack (early tbs, tail) and DVE does not.
            """
            t_sl = slice(tb * 512, (tb + 1) * 512)
            def wo_item(ob):
                def fn():
                    po = ps_acc.tile([128, 512], mybir.dt.float32, tag="acc",
                                     name="po")
                    for s in range(SLABS):
                        nc.tensor.matmul(po[:], WO[s][:, ob * 128:(ob + 1) * 128],
                                         OTNr_by[(tb, s)][:],
                                         start=(s == 0), stop=(s == SLABS - 1))
                    osb = osbp.tile([128, 512], F32R, tag="osb", name="osb")
                    if cast_on_act:
                        nc.scalar.copy(osb[:], po[:])
                    else:
                        nc.vector.tensor_copy(osb[:], po[:])
                    nc.sync.dma_start(outT[ob * 128:(ob + 1) * 128, t_sl], osb[:])
                return fn
            for ob in obs:
                fill_append(("wo", tb), wo_item(ob))

        # ---------- attention ----------
        def emit_att_slab(tb, s, units_left_ref):
            qc = tb
            nblk = 4 * qc + 4
            QTr_s = QTr_by[(tb, s)]
            ot0 = ps_ot.tile([65, 512], mybir.dt.float32, tag="ot0", name="ot0")
            ot1 = ps_ot.tile([65, 512], mybir.dt.float32, tag="ot1", name="ot1")
            pend = None   # (pt, o, b) awaiting PV emission

            def emit_pv(pt, o, b):
                nc.tensor.matmul(ot0[:, o:512], V0[:, b * 65: b * 65 + 65],
                                 pt[:, o:512],
                                 start=(b == 0), stop=(b == nblk - 1))
                nc.tensor.matmul(ot1[:, o:512], V1[:, b * 65: b * 65 + 65],
                                 pt[:, 512 + o:1024],
                                 start=(b == 0), stop=(b == nblk - 1))

            pend = []  # PV emission lags 2 units behind the score/exp stream
            for b in range(nblk):
                o = max(0, (b - 4 * qc) * 128)
                sc = ps_sc.tile([128, 1024], mybir.dt.float32, tag="sc", name="sc")
                # both halves of this s-block: row-tiles (0,0)/(64,0), adjacent
                # in PE order -> concurrent
                nc.tensor.matmul(sc[:, o:512], KT[0:64, b * 128:(b + 1) * 128],
                                 QTr_s[0:64, o:512], start=True, stop=True)
                nc.tensor.matmul(sc[:, 512 + o:1024], KT[64:128, b * 128:(b + 1) * 128],
                                 QTr_s[64:128, o:512], start=True, stop=True)
                pt = ptp.tile([128, 1024], F32R, tag="pt", name="pt")
                nc.scalar.activation(pt[:, o:1024], sc[:, o:1024], EXP)
                if b >= 4 * qc:  # diagonal block: zero the intra-block triangle
                    v = b - 4 * qc
                    nc.vector.tensor_mul(pt[:, o:1024], pt[:, o:1024],
                                         MSK[:, v * 1024 + o:(v + 1) * 1024])
                units_left_ref[0] -= 1
                pend.append((pt, o, b))
                # batch PVs in pairs so the PE sees 4-MM runs of each tiling
                # mode (scores 64-row, PV 128-row) -> half the mode switches
                if b % 2 == 1:
                    take_fill(max(1, units_left_ref[0] // 2))
                    if len(pend) >= 4:
                        for item in (pend.pop(0), pend.pop(0)):
                            if item[2] == 0:
                                flush_label(("V", tb))
                            emit_pv(*item)
            for item in pend:
                if item[2] == 0:
                    flush_label(("V", tb))
                emit_pv(*item)

            # ---- denominators + normalize: evacuate each PSUM accumulator to
            # SBUF with one copy (frees the ot slot for the next slab right
            # away), then run the recip/broadcast/normalize off the copy ----
            OTNr_s = rot.tile([128, 512], F32R, tag=f"otnr{s}", name=f"otnr{s}")
            OTNr_by[(tb, s)] = OTNr_s
            for h, oth in ((0, ot0), (1, ot1)):
                d = misc.tile([1, 512], F32, tag=f"d{h}", name=f"d{h}")
                nc.vector.tensor_copy(d[:], oth[64:65, :])
                osb_h = misc.tile([64, 512], F32, tag=f"osb{h}", name=f"osb{h}")
                nc.vector.tensor_copy(osb_h[:], oth[0:64, :])
                r = misc.tile([1, 512], F32, tag=f"r{h}", name=f"r{h}")
                nc.vector.reciprocal_approx_fast(r[:], d[:])
                bch = misc.tile([64, 512], F32, tag=f"bc{h}", name=f"bc{h}")
                nc.gpsimd.partition_broadcast(bch[:], r[:])
                nc.vector.tensor_mul(OTNr_s[h * 64:(h + 1) * 64, :],
                                     osb_h[:], bch[:])

        # ---------- main loop ----------
        # wo(tb) is split 8/8 across the next two tb windows so the exp-heavy
        # late chunks keep enough PE filler
        append_proj_items(0)
        for tb in range(TBLK):
            if tb >= 2:
                # pops first in this tb, freeing OTNr(tb-2) slots early
                append_wo_items(tb - 2, range(8, 16), cast_on_act=(tb <= 2))
            if tb == TBLK - 1:
                append_wo_items(tb - 1, range(8, 12))
            if tb + 1 < TBLK:
                append_proj_items(tb + 1)
            units_left = [4 * (4 * tb + 4)]
            for s in range(SLABS):
                flush_label(("X", tb))
                flush_label(("K", tb))
                flush_label(("V", tb))
                flush_label(("Q", tb, s))
                emit_att_slab(tb, s, units_left)
            if tb == TBLK - 1:
                # reserved for the tail: ready PE work that fills the last
                # slab's denominator-chain latency before wo(tb3) can start
                append_wo_items(tb - 1, range(12, 16), cast_on_act=True)
            # popped during tb+1 (or the tail for the last tb): ACT has slack
            # there except during tb3's exp-heavy attention
            append_wo_items(tb, range(0, 8),
                            cast_on_act=(tb <= 1 or tb == TBLK - 1))
        append_wo_items(TBLK - 1, range(8, 16), cast_on_act=True)
        flush_all()

    nc.compile()
    return nc


def _prep_inputs(x, freqs_cos, freqs_sin, wq, wk, wv, wo):
    """Build the 8 per-core input maps (host-side sharding + layout prep)."""
    x = np.asarray(x, dtype=np.float32)
    freqs_cos = np.asarray(freqs_cos, dtype=np.float32)
    freqs_sin = np.asarray(freqs_sin, dtype=np.float32)
    wq = np.asarray(wq, dtype=np.float32)
    wk = np.asarray(wk, dtype=np.float32)
    wv = np.asarray(wv, dtype=np.float32)
    wo = np.asarray(wo, dtype=np.float32)

    # de-interleave permutation within a head: [2j] then [2j+1]
    deint = np.concatenate([np.arange(0, HD, 2), np.arange(1, HD, 2)])

    # rope tables [128, T]: row r uses freq index r % 32; sign of sin flips
    # per 32-block (real-out blocks get -sin)
    cosT = freqs_cos.T  # [32, T]
    sinT = freqs_sin.T
    c4 = np.tile(cosT, (4, 1)).astype(np.float32)
    s4 = np.concatenate([-sinT, sinT, -sinT, sinT], axis=0).astype(np.float32)

    vones = np.zeros((128, 16 * 65), dtype=np.float32)
    vones[:, 64::65] = 1.0
    # block masks: variant v covers diagonal s-block at offset 128v vs q in
    # [0,512); duplicated for the two halves (cols 0:512 and 512:1024):
    # msk[p, v*1024 + h*512 + q] = 1 if (128v + p) <= q else 0
    msk = np.zeros((128, 4096), dtype=np.float32)
    p_ = np.arange(128)[:, None]
    q_ = np.arange(512)[None, :]
    for v in range(4):
        blk = (128 * v + p_) <= q_
        for h in range(2):
            msk[:, v * 1024 + h * 512:(v * 1024 + (h + 1) * 512)] = blk

    in_maps = []
    for core in range(8):
        b, g = divmod(core, 4)
        # local q head order: slab-major, (s, half) -> global head 8g + s + 4*half
        qheads = [8 * g + s + 4 * h for s in range(SLABS) for h in range(2)]
        kvheads = [2 * g, 2 * g + 1]

        wq_cols = np.concatenate([qh * HD + deint for qh in qheads])
        wk_cols = np.concatenate([kh * HD + deint for kh in kvheads])
        wv_cols = np.concatenate([np.arange(kh * HD, (kh + 1) * HD) for kh in kvheads])
        wo_rows = np.concatenate([np.arange(qh * HD, (qh + 1) * HD) for qh in qheads])

        def prearr(w):
            # [DIM, C] -> [128, KTILES*C]: element [p, k*C+c] = w[k*128+p, c],
            # matching the SBUF layout so the device DMA is contiguous
            C = w.shape[1]
            return w.reshape(KTILES, 128, C).transpose(1, 0, 2).reshape(128, KTILES * C)

        in_maps.append({
            "xT": _to_mm_dtype(x[b].T),
            "wq": _to_mm_dtype(prearr(wq[:, wq_cols] * (1.0 / np.sqrt(HD)))),
            "wk": _to_mm_dtype(prearr(wk[:, wk_cols])),
            "wv": _to_mm_dtype(prearr(wv[:, wv_cols])),
            "wo": _to_mm_dtype(wo[wo_rows, :]),
            "c4": _to_mm_dtype(c4),
            "s4": _to_mm_dtype(s4),
            "vones": _to_mm_dtype(vones),
            "msk": _to_mm_dtype(msk),
        })
    return in_maps


def kernel(x, freqs_cos, freqs_sin, wq, wk, wv, wo, _trace=False):
    from concourse.bass_utils import run_bass_kernel_spmd

    if "nc" not in _CACHE:
        _CACHE["nc"] = _build()
    nc = _CACHE["nc"]

    in_maps = _prep_inputs(x, freqs_cos, freqs_sin, wq, wk, wv, wo)
    res = run_bass_kernel_spmd(nc, in_maps, core_ids=list(range(8)), trace=_trace)
    _CACHE["last_result"] = res

    out = np.empty((B, T, DIM), dtype=np.float32)
    for b in range(B):
        acc = res.results[4 * b]["outT"].astype(np.float32)
        for g in range(1, 4):
            acc = acc + res.results[4 * b + g]["outT"].astype(np.float32)
        out[b] = acc.T
    return out


# revision 41
# speedup vs baseline: 1.0056x; 1.0019x over previous
"""GQA attention (B=2, T=2048, DIM=2048, NH=32, NKV=8, HD=64) with RoPE, causal,
on 8 TRN2 NeuronCores.

Sharding: data-parallel over B (2) x tensor-parallel over kv-head groups (4).
Core i handles batch i//4 and kv heads {2g, 2g+1} (g = i%4), i.e. q heads
8g..8g+8. wq/wk/wv column-parallel, wo row-parallel; host sums the 4 partial
outputs per batch.

Everything on-device is feature-major ("transposed"): x^T, Q^T, K^T are
[feature, t] so no on-device transposes are needed anywhere:
  QT[d,t] = wq^T x^T;  scoresT[s,q] = (KT slice)^T @ QT;  OT[d,q] = V^T @ PT;
  outT[o,t] = wo^T @ OT.  Host transposes the final [o,t] back to [t,o].

v2 structure (vs baseline):
- Attention inner unit is one 128-row s-block covering BOTH kv halves: the two
  score matmuls (K=64) write cols 0:512 / 512:1024 of one [128,1024] PSUM pair
  and auto-derive PE row-tiles (0,0)/(64,0), so being emission-adjacent they
  run CONCURRENTLY (2x score throughput). One 1024-col exp covers both halves.
- Projection / wo matmuls are interleaved as "filler" into the attention
  emission stream in program order, so the PE works on dense GEMMs while ACT
  streams the exps instead of stalling on the score->exp->PV chain.
- Denominator: ones-column on V gives row 64 of each PV accumulator; the two
  denom rows are DMA-gathered to one [2,512] tile, one reciprocal, gpsimd
  partition-broadcasts, and one multiply per half produce the normalized OTNr.
- ACT exp table preloaded at kernel start (off critical path).

Softmax: no max subtraction (|score| <~ 6 after the 1/8 scale folded into wq).
Causal mask: DVE multiply by a host-precomputed 0/1 tile on diagonal s-blocks
only (mask layout duplicated per half); fully-masked regions are trimmed from
the matmuls.

All matmul operands are fp16 (KERNEL_MM_DTYPE also allows bf16/f32r); fp32
PSUM accumulate.
"""

import numpy as np

B, T, DIM = 2, 2048, 2048
NH, NKV, HD = 32, 8, 64
G = 4            # tensor-parallel groups
QH = NH // G     # 8 local q heads
SLABS = 4
KTILES = DIM // 128
TBLK = T // 512

_CACHE = {}
import os as _os
_MM_DTYPE = _os.environ.get("KERNEL_MM_DTYPE", "fp16")


def _to_mm_dtype(x: np.ndarray) -> np.ndarray:
    if _MM_DTYPE == "bf16":
        import ml_dtypes
        return np.ascontiguousarray(x, dtype=np.float32).astype(ml_dtypes.bfloat16)
    if _MM_DTYPE == "fp16":
        return np.ascontiguousarray(x, dtype=np.float32).astype(np.float16)
    return _round_f32r(x)


def _round_f32r(x: np.ndarray) -> np.ndarray:
    """Round f32 to the float32r grid (11 mantissa bits, round-to-nearest-even)."""
    x = np.ascontiguousarray(x, dtype=np.float32)
    xi = x.view(np.uint32).copy()
    shift = 12  # keep 11 mantissa bits
    lsb = (xi >> shift) & 1
    xi = (xi + ((1 << (shift - 1)) - 1) + lsb) & np.uint32(~((1 << shift) - 1) & 0xFFFFFFFF)
    return xi.view(np.float32)


def _build():
    import concourse.bass as bass
    import concourse.mybir as mybir
    import concourse.tile as tile
    from concourse import bacc

    F32 = mybir.dt.float32
    F32R = {"bf16": mybir.dt.bfloat16, "fp16": mybir.dt.float16,
            "f32r": mybir.dt.float32r}[_MM_DTYPE]
    EXP = mybir.ActivationFunctionType.Exp

    nc = bacc.Bacc("TRN2", target_bir_lowering=False, debug=False, num_devices=8)

    xT = nc.dram_tensor("xT", [DIM, T], F32R, kind="ExternalInput").ap()
    # weights arrive host-pre-arranged to the SBUF layout [p, k*C + c] so the
    # loads are contiguous 128-row DMAs instead of thousands of tiny
    # rearrange descriptors (the old WK/WV triggers alone cost ~1.4us each
    # and delayed the first matmul to ~17us)
    wq = nc.dram_tensor("wq", [128, KTILES * QH * HD], F32R, kind="ExternalInput").ap()
    wk = nc.dram_tensor("wk", [128, KTILES * 2 * HD], F32R, kind="ExternalInput").ap()
    wv = nc.dram_tensor("wv", [128, KTILES * 2 * HD], F32R, kind="ExternalInput").ap()
    wo = nc.dram_tensor("wo", [QH * HD, DIM], F32R, kind="ExternalInput").ap()
    c4 = nc.dram_tensor("c4", [128, T], F32R, kind="ExternalInput").ap()
    s4 = nc.dram_tensor("s4", [128, T], F32R, kind="ExternalInput").ap()
    vones = nc.dram_tensor("vones", [128, 16 * 65], F32R, kind="ExternalInput").ap()
    msk = nc.dram_tensor("msk", [128, 4096], F32R, kind="ExternalInput").ap()
    outT = nc.dram_tensor("outT", [DIM, T], F32R, kind="ExternalOutput").ap()

    from contextlib import ExitStack

    with tile.TileContext(nc) as tc, ExitStack() as ctx:
        # ---------- persistent tiles ----------
        pers = ctx.enter_context(tc.tile_pool(name="pers", bufs=1))
        KT = pers.tile([128, T], F32R, tag="kt", name="kt")
        V0 = pers.tile([128, 16 * 65], F32R, tag="v0", name="v0")
        V1 = pers.tile([128, 16 * 65], F32R, tag="v1", name="v1")
        MSK = pers.tile([128, 4096], F32R, tag="msk", name="msk_sb")
        C4 = pers.tile([128, T], F32R, tag="c4", name="c4_sb")
        S4 = pers.tile([128, T], F32R, tag="s4", name="s4_sb")
        WQ = pers.tile([128, KTILES * 512], F32R, tag="wq", name="wq_sb")
        WK = pers.tile([128, KTILES * 128], F32R, tag="wk", name="wk_sb")
        WV = pers.tile([128, KTILES * 128], F32R, tag="wv", name="wv_sb")
        WO = [pers.tile([128, T], F32R, tag=f"wo{s}", name=f"wo{s}") for s in range(SLABS)]

        rot = ctx.enter_context(tc.tile_pool(name="rot", bufs=3))
        work = ctx.enter_context(tc.tile_pool(name="work", bufs=3))
        ptp = ctx.enter_context(tc.tile_pool(name="ptp", bufs=6))
        misc = ctx.enter_context(tc.tile_pool(name="misc", bufs=2))
        osbp = ctx.enter_context(tc.tile_pool(name="osbp", bufs=6))
        xtp = ctx.enter_context(tc.tile_pool(name="xt", bufs=2))
        ps_acc = ctx.enter_context(tc.tile_pool(name="ps_acc", bufs=2, space="PSUM"))
        ps_sc = ctx.enter_context(tc.tile_pool(name="ps_sc", bufs=2, space="PSUM"))
        ps_ot = ctx.enter_context(tc.tile_pool(name="ps_ot", bufs=1, space="PSUM"))

        # ---------- initial DMAs ----------
        def load_xts(tb, names):
            t_sl = slice(tb * 512, (tb + 1) * 512)
            xts = []
            src = xT.rearrange("(k p) t -> p k t", p=128)[:, :, t_sl]
            for k in range(KTILES):
                xts.append(xtp.tile([128, 512], F32R, tag=f"x{k}",
                                    name=f"{names}_{k}"))
            for k in range(KTILES):
                nc.sync.dma_start(xts[k][:], src[:, k])
            return xts

        xts_by_tb = {0: load_xts(0, "xt0")}
        # small operands needed by tb0's K-proj / attention go before the 2MB WQ
        nc.gpsimd.dma_start(WK[:], wk)
        nc.gpsimd.dma_start(WV[:], wv)
        nc.gpsimd.dma_start(C4[:, 0:512], c4[:, 0:512])
        nc.gpsimd.dma_start(S4[:, 0:512], s4[:, 0:512])
        nc.gpsimd.dma_start(V0[:], vones[:])
        nc.gpsimd.dma_start(V1[:], vones[:])
        nc.gpsimd.dma_start(MSK[:], msk[:])
        for c in range(4):
            nc.scalar.dma_start(WQ[:, c * 2048:(c + 1) * 2048],
                                wq[:, c * 2048:(c + 1) * 2048])
        nc.gpsimd.dma_start(C4[:, 512:2048], c4[:, 512:2048])
        nc.gpsimd.dma_start(S4[:, 512:2048], s4[:, 512:2048])
        for s in range(SLABS):
            nc.scalar.dma_start(WO[s][:], wo[s * 128:(s + 1) * 128, :])

        # preload the exp table set while DMAs stream (first real exp would
        # otherwise pay the ~2.7us ACT_TABLE_LOAD on the critical path)
        warm = misc.tile([1, 8], F32, tag="warm", name="warm")
        nc.scalar.activation(warm[:], C4[0:1, 0:8], EXP)

        # ---------- emission machinery: labeled filler queue ----------
        QTr_by = {}    # (tb, s) -> QTr tile, written when the Q-proj item pops
        OTNr_by = {}   # (tb, s) -> OTNr tile, written by emit_att_slab

        filler_q = []            # list of (label, fn)
        label_left = {}          # label -> count not yet emitted

        def fill_append(label, fn):
            filler_q.append((label, fn))
            label_left[label] = label_left.get(label, 0) + 1

        def _pop_one():
            label, fn = filler_q.pop(0)
            label_left[label] -= 1
            fn()

        def flush_label(label):
            while label_left.get(label, 0) > 0:
                _pop_one()

        def take_fill(units_left):
            k = (len(filler_q) + units_left - 1) // units_left if units_left > 0 else len(filler_q)
            k = min(k, 4, len(filler_q))
            for _ in range(k):
                _pop_one()

        def flush_all():
            while filler_q:
                _pop_one()

        # ---------- projection emission ----------
        def rope_tail(ps, dst, t_sl):
            """dst = ps*C4 + swap(ps)*S4 (swap via gpsimd DMA). ps is a psum tile."""
            q_sb = work.tile([128, 512], F32, tag="qsb", name="qsb")
            nc.vector.tensor_copy(q_sb[:], ps[:])
            q_sw = work.tile([128, 512], F32, tag="qsw", name="qsw")
            for o in (0, 64):
                nc.gpsimd.dma_start(q_sw[o:o + 32, :], q_sb[o + 32:o + 64, :])
                nc.gpsimd.dma_start(q_sw[o + 32:o + 64, :], q_sb[o:o + 32, :])
            m1 = work.tile([128, 512], F32, tag="m1", name="m1")
            nc.vector.tensor_mul(m1[:], ps[:], C4[:, t_sl])
            m2 = work.tile([128, 512], F32, tag="m2", name="m2")
            nc.vector.tensor_mul(m2[:], q_sw[:], S4[:, t_sl])
            nc.vector.tensor_add(dst, m1[:], m2[:])

        def append_proj_items(tb):
            """Queue tb's x loads + K/V/Q projection groups as filler items."""
            t_sl = slice(tb * 512, (tb + 1) * 512)

            def do_load_x():
                xts_by_tb[tb] = load_xts(tb, f"xt{tb}")
            if tb not in xts_by_tb:
                fill_append(("X", tb), do_load_x)

            # K projection: 16 accumulating MMs in 4-MM chunks + rope into
            # KT (8-MM chunks measured slower: coarser filler starves ACT)
            csz = 4
            nch = KTILES // csz
            kps = {}
            def k_chunk(c):
                def fn():
                    if c == 0:
                        kps["ps"] = ps_acc.tile([128, 512], mybir.dt.float32,
                                                tag="acc", name="pk")
                    ps = kps["ps"]
                    xts = xts_by_tb[tb]
                    for k in range(c * csz, (c + 1) * csz):
                        nc.tensor.matmul(ps[:], WK[:, k * 128:(k + 1) * 128],
                                         xts[k][:], start=(k == 0),
                                         stop=(k == KTILES - 1))
                    if c == nch - 1:
                        rope_tail(ps, KT[:, t_sl], t_sl)
                return fn

            # Q projections: per slab 16 MMs in chunks + rope into QTr
            def make_q_chunk(s):
                qps = {}
                def q_chunk(c):
                    def fn():
                        if c == 0:
                            qps["ps"] = ps_acc.tile([128, 512], mybir.dt.float32,
                                                    tag="acc", name="pq")
                        ps = qps["ps"]
                        xts = xts_by_tb[tb]
                        for k in range(c * csz, (c + 1) * csz):
                            nc.tensor.matmul(
                                ps[:], WQ[:, k * 512 + s * 128: k * 512 + (s + 1) * 128],
                                xts[k][:], start=(k == 0), stop=(k == KTILES - 1))
                        if c == nch - 1:
                            dst_t = rot.tile([128, 512], F32R, tag=f"qtr{s}",
                                             name=f"qtr{s}")
                            QTr_by[(tb, s)] = dst_t
                            rope_tail(ps, dst_t[:], t_sl)
                    return fn
                return q_chunk

            if tb == 0:
                # head: interleave K and Q0 chunks so PE can start as soon as
                # the first x k-tiles land instead of waiting for all 16
                q0 = make_q_chunk(0)
                for c in range(nch):
                    fill_append(("K", tb), k_chunk(c))
                    fill_append(("Q", tb, 0), q0(c))
            else:
                for c in range(nch):
                    fill_append(("K", tb), k_chunk(c))

            # V projection: 4 items (one per 128-t block)
            def v_item(i):
                def fn():
                    sbi = tb * 4 + i
                    xts = xts_by_tb[tb]
                    pv = ps_acc.tile([128, 128], mybir.dt.float32, tag="acc",
                                     name="pv", padded_shape=[128, 512])
                    for k in range(KTILES):
                        nc.tensor.matmul(pv[:], xts[k][:, i * 128:(i + 1) * 128],
                                         WV[:, k * 128:(k + 1) * 128],
                                         start=(k == 0), stop=(k == KTILES - 1))
                    nc.vector.tensor_copy(V0[:, sbi * 65: sbi * 65 + 64], pv[:, 0:64])
                    nc.vector.tensor_copy(V1[:, sbi * 65: sbi * 65 + 64], pv[:, 64:128])
                return fn
            for i in range(4):
                fill_append(("V", tb), v_item(i))

            for s in range(0 if tb != 0 else 1, SLABS):
                qc_ = make_q_chunk(s)
                for c in range(nch):
                    fill_append(("Q", tb, s), qc_(c))

        def append_wo_items(tb, obs, cast_on_act=False):
            """Queue some of tb's output projection ob-groups as filler items.

            cast_on_act routes the PSUM->SBUF cast to the Scalar engine for
            windows where ACT has slack (early tbs, tail) and DVE does not.
            """
            t_sl = slice(tb * 512, (tb + 1) * 512)
            def wo_item(ob):
                def fn():
                    po = ps_acc.tile([128, 512], mybir.dt.float32, tag="acc",
                                     name="po")
                    for s in range(SLABS):
                        nc.tensor.matmul(po[:], WO[s][:, ob * 128:(ob + 1) * 128],
                                         OTNr_by[(tb, s)][:],
                                         start=(s == 0), stop=(s == SLABS - 1))
                    osb = osbp.tile([128, 512], F32R, tag="osb", name="osb")
                    if cast_on_act:
                        nc.scalar.copy(osb[:], po[:])
                    else:
                        nc.vector.tensor_copy(osb[:], po[:])
                    nc.sync.dma_start(outT[ob * 128:(ob + 1) * 128, t_sl], osb[:])
                return fn
            for ob in obs:
                fill_append(("wo", tb), wo_item(ob))

        # ---------- attention ----------
        def emit_att_slab(tb, s, units_left_ref):
            qc = tb
            nblk = 4 * qc + 4
            QTr_s = QTr_by[(tb, s)]
            ot0 = ps_ot.tile([65, 512], mybir.dt.float32, tag="ot0", name="ot0")
            ot1 = ps_ot.tile([65, 512], mybir.dt.float32, tag="ot1", name="ot1")
            pend = None   # (pt, o, b) awaiting PV emission

            def emit_pv(pt, o, b):
                nc.tensor.matmul(ot0[:, o:512], V0[:, b * 65: b * 65 + 65],
                                 pt[:, o:512],
                                 start=(b == 0), stop=(b == nblk - 1))
                nc.tensor.matmul(ot1[:, o:512], V1[:, b * 65: b * 65 + 65],
                                 pt[:, 512 + o:1024],
                                 start=(b == 0), stop=(b == nblk - 1))

            pend = []  # PV emission lags 2 units behind the score/exp stream
            for b in range(nblk):
                o = max(0, (b - 4 * qc) * 128)
                sc = ps_sc.tile([128, 1024], mybir.dt.float32, tag="sc", name="sc")
                # both halves of this s-block: row-tiles (0,0)/(64,0), adjacent
                # in PE order -> concurrent
                nc.tensor.matmul(sc[:, o:512], KT[0:64, b * 128:(b + 1) * 128],
                                 QTr_s[0:64, o:512], start=True, stop=True)
                nc.tensor.matmul(sc[:, 512 + o:1024], KT[64:128, b * 128:(b + 1) * 128],
                                 QTr_s[64:128, o:512], start=True, stop=True)
                pt = ptp.tile([128, 1024], F32R, tag="pt", name="pt")
                nc.scalar.activation(pt[:, o:1024], sc[:, o:1024], EXP)
                if b >= 4 * qc:  # diagonal block: zero the intra-block triangle
                    v = b - 4 * qc
                    nc.vector.tensor_mul(pt[:, o:1024], pt[:, o:1024],
                                         MSK[:, v * 1024 + o:(v + 1) * 1024])
                units_left_ref[0] -= 1
                pend.append((pt, o, b))
                # batch PVs in pairs so the PE sees 4-MM runs of each tiling
                # mode (scores 64-row, PV 128-row) -> half the mode switches
                if b % 2 == 1:
                    take_fill(max(1, units_left_ref[0] // 2))
                    if len(pend) >= 4:
                        for item in (pend.pop(0), pend.pop(0)):
                            if item[2] == 0:
                                flush_label(("V", tb))
                            emit_pv(*item)
            for item in pend:
                if item[2] == 0:
                    flush_label(("V", tb))
                emit_pv(*item)

            # ---- denominators + normalize: evacuate each PSUM accumulator to
            # SBUF with one copy (frees the ot slot for the next slab right
            # away), then run the recip/broadcast/normalize off the copy ----
            OTNr_s = rot.tile([128, 512], F32R, tag=f"otnr{s}", name=f"otnr{s}")
            OTNr_by[(tb, s)] = OTNr_s
            for h, oth in ((0, ot0), (1, ot1)):
                d = misc.tile([1, 512], F32, tag=f"d{h}", name=f"d{h}")
                nc.vector.tensor_copy(d[:], oth[64:65, :])
                osb_h = misc.tile([64, 512], F32, tag=f"osb{h}", name=f"osb{h}")
                nc.vector.tensor_copy(osb_h[:], oth[0:64, :])
                r = misc.tile([1, 512], F32, tag=f"r{h}", name=f"r{h}")
                nc.vector.reciprocal_approx_fast(r[:], d[:])
                bch = misc.tile([64, 512], F32, tag=f"bc{h}", name=f"bc{h}")
                nc.gpsimd.partition_broadcast(bch[:], r[:])
                nc.vector.tensor_mul(OTNr_s[h * 64:(h + 1) * 64, :],
                                     osb_h[:], bch[:])

        # ---------- main loop ----------
        # wo(tb) is split 8/8 across the next two tb windows so the exp-heavy
        # late chunks keep enough PE filler
        append_proj_items(0)
        for tb in range(TBLK):
            if tb >= 2:
                # pops first in this tb, freeing OTNr(tb-2) slots early
                append_wo_items(tb - 2, range(8, 16), cast_on_act=(tb <= 2))
            if tb == TBLK - 1:
                append_wo_items(tb - 1, range(8, 16))
            if tb + 1 < TBLK:
                append_proj_items(tb + 1)
            units_left = [4 * (4 * tb + 4)]
            for s in range(SLABS):
                flush_label(("X", tb))
                flush_label(("K", tb))
                flush_label(("V", tb))
                flush_label(("Q", tb, s))
                emit_att_slab(tb, s, units_left)
            # popped during tb+1 (or the tail for the last tb): ACT has slack
            # there except during tb3's exp-heavy attention
            append_wo_items(tb, range(0, 8),
                            cast_on_act=(tb <= 1 or tb == TBLK - 1))
        append_wo_items(TBLK - 1, range(8, 16), cast_on_act=True)
        flush_all()

    nc.compile()
    return nc


def _prep_inputs(x, freqs_cos, freqs_sin, wq, wk, wv, wo):
    """Build the 8 per-core input maps (host-side sharding + layout prep)."""
    x = np.asarray(x, dtype=np.float32)
    freqs_cos = np.asarray(freqs_cos, dtype=np.float32)
    freqs_sin = np.asarray(freqs_sin, dtype=np.float32)
    wq = np.asarray(wq, dtype=np.float32)
    wk = np.asarray(wk, dtype=np.float32)
    wv = np.asarray(wv, dtype=np.float32)
    wo = np.asarray(wo, dtype=np.float32)

    # de-interleave permutation within a head: [2j] then [2j+1]
    deint = np.concatenate([np.arange(0, HD, 2), np.arange(1, HD, 2)])

    # rope tables [128, T]: row r uses freq index r % 32; sign of sin flips
    # per 32-block (real-out blocks get -sin)
    cosT = freqs_cos.T  # [32, T]
    sinT = freqs_sin.T
    c4 = np.tile(cosT, (4, 1)).astype(np.float32)
    s4 = np.concatenate([-sinT, sinT, -sinT, sinT], axis=0).astype(np.float32)

    vones = np.zeros((128, 16 * 65), dtype=np.float32)
    vones[:, 64::65] = 1.0
    # block masks: variant v covers diagonal s-block at offset 128v vs q in
    # [0,512); duplicated for the two halves (cols 0:512 and 512:1024):
    # msk[p, v*1024 + h*512 + q] = 1 if (128v + p) <= q else 0
    msk = np.zeros((128, 4096), dtype=np.float32)
    p_ = np.arange(128)[:, None]
    q_ = np.arange(512)[None, :]
    for v in range(4):
        blk = (128 * v + p_) <= q_
        for h in range(2):
            msk[:, v * 1024 + h * 512:(v * 1024 + (h + 1) * 512)] = blk

    in_maps = []
    for core in range(8):
        b, g = divmod(core, 4)
        # local q head order: slab-major, (s, half) -> global head 8g + s + 4*half
        qheads = [8 * g + s + 4 * h for s in range(SLABS) for h in range(2)]
        kvheads = [2 * g, 2 * g + 1]

        wq_cols = np.concatenate([qh * HD + deint for qh in qheads])
        wk_cols = np.concatenate([kh * HD + deint for kh in kvheads])
        wv_cols = np.concatenate([np.arange(kh * HD, (kh + 1) * HD) for kh in kvheads])
        wo_rows = np.concatenate([np.arange(qh * HD, (qh + 1) * HD) for qh in qheads])

        def prearr(w):
            # [DIM, C] -> [128, KTILES*C]: element [p, k*C+c] = w[k*128+p, c],
            # matching the SBUF layout so the device DMA is contiguous
            C = w.shape[1]
            return w.reshape(KTILES, 128, C).transpose(1, 0, 2).reshape(128, KTILES * C)

        in_maps.append({
            "xT": _to_mm_dtype(x[b].T),
            "wq": _to_mm_dtype(prearr(wq[:, wq_cols] * (1.0 / np.sqrt(HD)))),
            "wk": _to_mm_dtype(prearr(wk[:, wk_cols])),
            "wv": _to_mm_dtype(prearr(wv[:, wv_cols])),
            "wo": _to_mm_dtype(wo[wo_rows, :]),
            "c4": _to_mm_dtype(c4),
            "s4": _to_mm_dtype(s4),
            "vones": _to_mm_dtype(vones),
            "msk": _to_mm_dtype(msk),
        })
    return in_maps


def kernel(x, freqs_cos, freqs_sin, wq, wk, wv, wo, _trace=False):
    from concourse.bass_utils import run_bass_kernel_spmd

    if "nc" not in _CACHE:
        _CACHE["nc"] = _build()
    nc = _CACHE["nc"]

    in_maps = _prep_inputs(x, freqs_cos, freqs_sin, wq, wk, wv, wo)
    res = run_bass_kernel_spmd(nc, in_maps, core_ids=list(range(8)), trace=_trace)
    _CACHE["last_result"] = res

    out = np.empty((B, T, DIM), dtype=np.float32)
    for b in range(B):
        acc = res.results[4 * b]["outT"].astype(np.float32)
        for g in range(1, 4):
            acc = acc + res.results[4 * b + g]["outT"].astype(np.float32)
        out[b] = acc.T
    return out
